# revision 2
# baseline (speedup 1.0000x reference)
"""Trainium2 Bass kernel for nn_Encoder_84069689852144 (GAT encoder pair + AE).

Self-contained: takes FULL inputs, shards across 8 NeuronCores internally,
returns FULL outputs (x_in, x_out, x_self, z_self_re).

Strategy (per core, SPMD one program, per-core data via in_maps):
  - Destination-node sharding: core k owns dst nodes [k*1250, (k+1)*1250),
    edges partitioned by dst, sorted, bucketed into 128-edge chunks per
    128-dst-node tile (host side).
  - GAT layer = on-the-fly: per chunk, dma_gather(transpose) pulls x[src]^T
    as a ready matmul lhsT; PE computes h(+a_s) into PSUM; attention weight
    g=exp(leaky(a_s+a_d)) (max-free softmax, exactly equivalent); DVE/ACT
    scale-evacuate to fp16 Xg; PE scatter-adds via a host-built 0/1 selection
    matrix S^T into a PSUM accumulator per dst tile; finalize divides by the
    segment sum, head-means (1/H folded into W), biases, ELU.
  - Between GAT layers: AllGather of z1 (fp16) across the 8 cores.
  - a_d tables per node built on device; gathered per-edge by dst index.
  - AE runs feature-major (no transposes between layers), PE-transposed at
    the end to node-major.
"""
import numpy as np

import concourse.bass as bass
import concourse.bacc as bacc
import concourse.mybir as mybir
import concourse.tile as tile
from concourse.bass_utils import run_bass_kernel_spmd

N = 10000
INF = 128
H = 32
C1 = 64
C2 = 32
NCORES = 8
NP = N // NCORES          # 1250 own dst nodes per core
NPAD = 1280               # padded own rows (10 tiles of 128)
NTILES = NPAD // 128
BN_EPS = 1e-5

F16 = mybir.dt.float16
F32 = mybir.dt.float32
I16 = mybir.dt.int16
AF = mybir.ActivationFunctionType
ALU = mybir.AluOpType
ENCS = ("i", "o")

_cache: dict = {}
LAST_RESULT = None        # BassKernelResults of the most recent run


# ----------------------------------------------------------------- host prep

def _perm_cols(c):
    """new col (cc*H + h) <- old col (h*c + cc)  (c-major layout)"""
    cc, hh = np.meshgrid(np.arange(c), np.arange(H), indexing="ij")
    return (hh * c + cc).reshape(-1)


def _fold_gat(p, c):
    W = np.asarray(p["W"], np.float64)
    a_s = np.asarray(p["a_src"], np.float64)
    a_d = np.asarray(p["a_dst"], np.float64)
    Wr = W.reshape(W.shape[0], H, c)
    WAs = np.einsum("ihc,hc->ih", Wr, a_s)
    WAd = np.einsum("ihc,hc->ih", Wr, a_d)
    Wp = W[:, _perm_cols(c)] / H          # 1/H (head mean) folded in
    b = np.asarray(p["b"], np.float64)
    return Wp, WAs, WAd, b


def _wrap16(a):
    """[M] int array -> dma_gather idx layout [128, M//16] int16."""
    m = a.shape[0]
    w = a.reshape(m // 16, 16).T
    return np.tile(w, (8, 1)).astype(np.int16)


def _prep_edges(src, dst, nch=None):
    """Per-core edge tables for one encoder direction."""
    order = np.argsort(dst, kind="stable")
    s_s = src[order].astype(np.int64)
    d_s = dst[order].astype(np.int64)
    core = d_s // NP
    local = d_s - core * NP
    t = local // 128
    key = core * NTILES + t
    counts = np.bincount(key, minlength=NCORES * NTILES)
    nch_req = int(np.ceil(counts.max() / 128))
    if nch is None:
        nch = nch_req
    assert nch >= nch_req
    cap = nch * 128
    nchunks = NTILES * nch
    starts = np.zeros(NCORES * NTILES, np.int64)
    starts[1:] = np.cumsum(counts)[:-1]
    within = np.arange(len(order)) - starts[key]
    slot = (key % NTILES) * cap + within

    srcg = np.zeros((NCORES, NTILES * cap), np.int64)
    src2 = np.zeros_like(srcg)
    dstl = np.zeros_like(srcg)
    st = np.zeros((NCORES, nchunks * 128, 128), np.float16)
    srcg[core, slot] = s_s
    src2[core, slot] = (s_s // NP) * NPAD + s_s % NP
    dstl[core, slot] = local
    st[core, slot, local - t * 128] = 1.0
    return nch, {
        "srcg": np.stack([_wrap16(srcg[k]) for k in range(NCORES)]),
        "src2": np.stack([_wrap16(src2[k]) for k in range(NCORES)]),
        "dstl": np.stack([_wrap16(dstl[k]) for k in range(NCORES)]),
        "st": st,
    }


def _prep_inputs(x, edge_index, params):
    x32 = np.asarray(x, np.float32)
    ei = np.asarray(edge_index).astype(np.int64)
    shared = {"x16": x32.astype(np.float16)}

    for e, (p1, p2) in (("i", (params["gin1"], params["gin2"])),
                        ("o", (params["gout1"], params["gout2"]))):
        Wp1, WAs1, WAd1, b1 = _fold_gat(p1, C1)
        Wp2, WAs2, WAd2, b2 = _fold_gat(p2, C2)
        shared[f"w1c_{e}"] = np.concatenate([Wp1, WAs1], 1).astype(np.float16)
        w2c = np.concatenate([Wp2, WAs2], 1)              # [64, 1056]
        shared[f"w2c_{e}"] = np.concatenate(
            [w2c, np.zeros((64, 1056))], 0).astype(np.float16)
        shared[f"wad1_{e}"] = WAd1.astype(np.float16)      # [128, 32]
        shared[f"wad2_{e}"] = np.concatenate(
            [WAd2, np.zeros((64, H))], 0).astype(np.float16)  # [128, 32]
        shared[f"b1r_{e}"] = np.broadcast_to(b1, (128, C1)).astype(np.float32)
        shared[f"b2r_{e}"] = np.broadcast_to(b2, (128, C2)).astype(np.float32)

    ae = params["ae"]
    sbn = 1.0 / np.sqrt(1.0 + BN_EPS)
    shared["w1e"] = np.asarray(ae["w1"], np.float16)            # [128, 64]
    shared["w2e"] = np.asarray(ae["w2"], np.float16)            # [64, 32]
    shared["dw1e"] = np.asarray(ae["dw1"], np.float16)          # [32, 64]
    shared["dw2e"] = np.asarray(ae["dw2"], np.float16)          # [64, 128]
    aev = np.zeros((128, 8), np.float32)
    aev[0:64, 0] = np.asarray(ae["b1"])
    aev[0:64, 1] = np.asarray(ae["g1"]) * sbn
    aev[0:64, 2] = np.asarray(ae["be1"])
    aev[0:32, 3] = np.asarray(ae["b2"])
    aev[0:32, 4] = np.asarray(ae["g2"]) * sbn
    aev[0:32, 5] = np.asarray(ae["be2"])
    aev[0:64, 6] = np.asarray(ae["db1"])
    aev[0:128, 7] = np.asarray(ae["db2"])
    shared["aev"] = aev
    shared["ident"] = np.eye(128, dtype=np.float32)

    nch_i, tab_i = _prep_edges(ei[0], ei[1])
    nch_o, tab_o = _prep_edges(ei[1], ei[0])
    nch = max(nch_i, nch_o)
    if nch_i < nch:
        _, tab_i = _prep_edges(ei[0], ei[1], nch)
    if nch_o < nch:
        _, tab_o = _prep_edges(ei[1], ei[0], nch)

    percore = []
    for k in range(NCORES):
        m = {}
        for e, tab in (("i", tab_i), ("o", tab_o)):
            m[f"srcg_{e}"] = tab["srcg"][k]
            m[f"src2_{e}"] = tab["src2"][k]
            m[f"dstl_{e}"] = tab["dstl"][k]
            m[f"st_{e}"] = tab["st"][k]
        own1 = k * NP + np.minimum(np.arange(NPAD), NP - 1)
        own2 = k * NPAD + np.arange(NPAD)
        m["own1"] = _wrap16(own1)
        m["own2"] = _wrap16(own2)
        percore.append(m)
    return nch, shared, percore


# -------------------------------------------------------------- bass builder

def _bc(ap2d, rep):
    """[P, W] AP -> [P, rep, W] with step-0 (broadcast) middle dim."""
    return bass.AP(ap2d.tensor, ap2d.offset, [ap2d.ap[0], [0, rep], ap2d.ap[-1]])


def _build(nch):
    nchunks = NTILES * nch
    nc = bacc.Bacc("TRN2", target_bir_lowering=False, debug=False,
                   num_devices=NCORES)

    def din(name, shape, dt):
        return nc.dram_tensor(name, shape, dt, kind="ExternalInput")

    x16d = din("x16", [N, INF], F16)
    w1cd = {e: din(f"w1c_{e}", [128, 2080], F16) for e in ENCS}
    w2cd = {e: din(f"w2c_{e}", [128, 1056], F16) for e in ENCS}
    wad1d = {e: din(f"wad1_{e}", [128, 32], F16) for e in ENCS}
    wad2d = {e: din(f"wad2_{e}", [128, 32], F16) for e in ENCS}
    b1rd = {e: din(f"b1r_{e}", [128, C1], F32) for e in ENCS}
    b2rd = {e: din(f"b2r_{e}", [128, C2], F32) for e in ENCS}
    w1ed = din("w1e", [128, 64], F16)
    w2ed = din("w2e", [64, 32], F16)
    dw1ed = din("dw1e", [32, 64], F16)
    dw2ed = din("dw2e", [64, 128], F16)
    aevd = din("aev", [128, 8], F32)
    identd = din("ident", [128, 128], F32)
    srcgd = {e: din(f"srcg_{e}", [128, nchunks * 8], I16) for e in ENCS}
    src2d = {e: din(f"src2_{e}", [128, nchunks * 8], I16) for e in ENCS}
    dstld = {e: din(f"dstl_{e}", [128, nchunks * 8], I16) for e in ENCS}
    std = {e: din(f"st_{e}", [nchunks * 128, 128], F16) for e in ENCS}
    own1d = din("own1", [128, NTILES * 8], I16)
    own2d = din("own2", [128, NTILES * 8], I16)

    xshd = {"i": nc.dram_tensor("xin_sh", [NPAD, 96], F32, kind="ExternalOutput"),
            "o": nc.dram_tensor("xout_sh", [NPAD, 96], F32, kind="ExternalOutput")}
    xselfd = nc.dram_tensor("xself_sh", [NPAD, 96], F32, kind="ExternalOutput")
    zred = nc.dram_tensor("zre_sh", [NPAD, 128], F32, kind="ExternalOutput")

    with tile.TileContext(nc) as tc:
        with (
            tc.tile_pool(name="const", bufs=1) as cp,
            tc.tile_pool(name="dram", bufs=1, space="DRAM") as dp,
        ):
            def load_const(dram_t, shape, dt):
                t = cp.tile(shape, dt, tag=dram_t.name + "_sb")
                nc.sync.dma_start(t[:], dram_t[:])
                return t

            w1s = {e: load_const(w1cd[e], [128, 2080], F16) for e in ENCS}
            w2s = {e: load_const(w2cd[e], [128, 1056], F16) for e in ENCS}
            wad1s = {e: load_const(wad1d[e], [128, 32], F16) for e in ENCS}
            wad2s = {e: load_const(wad2d[e], [128, 32], F16) for e in ENCS}
            b1rs = {e: load_const(b1rd[e], [128, C1], F32) for e in ENCS}
            b2rs = {e: load_const(b2rd[e], [128, C2], F32) for e in ENCS}
            w1es = load_const(w1ed, [128, 64], F16)
            w2es = load_const(w2ed, [64, 32], F16)
            dw1es = load_const(dw1ed, [32, 64], F16)
            dw2es = load_const(dw2ed, [64, 128], F16)
            aevs = load_const(aevd, [128, 8], F32)
            idents = load_const(identd, [128, 128], F32)
            srcgs = {e: load_const(srcgd[e], [128, nchunks * 8], I16) for e in ENCS}
            src2s = {e: load_const(src2d[e], [128, nchunks * 8], I16) for e in ENCS}
            dstls = {e: load_const(dstld[e], [128, nchunks * 8], I16) for e in ENCS}
            own1s = load_const(own1d, [128, NTILES * 8], I16)
            own2s = load_const(own2d, [128, NTILES * 8], I16)

            ad1 = {e: dp.tile([NPAD, 128], F16, tag=f"ad1_{e}", name=f"ad1_{e}") for e in ENCS}
            ad2 = {e: dp.tile([NPAD, 128], F16, tag=f"ad2_{e}", name=f"ad2_{e}") for e in ENCS}
            z1own = {e: dp.tile([NPAD, 128], F16, tag=f"z1own_{e}", name=f"z1own_{e}") for e in ENCS}
            z1all = {e: dp.tile([NCORES * NPAD, 128], F16, tag=f"z1all_{e}", name=f"z1all_{e}")
                     for e in ENCS}

            def gath_t(pool, table_ap, idx_sb, q, tag):
                """gather-transpose 128 rows of 128 fp16 -> [128, 128] lhsT"""
                t = pool.tile([128, 128], F16, tag=tag)
                nc.gpsimd.dma_gather(
                    out_ap=t[:].rearrange("p (a n) -> p a n", a=1),
                    in_ap=table_ap,
                    idxs_ap=idx_sb[:, q * 8:(q + 1) * 8],
                    num_idxs=128, num_idxs_reg=128, elem_size=128,
                    transpose=True)
                return t

            def gath_r(pool, table_ap, idx_sb, q, tag):
                """plain gather: rows on partitions -> [128, 128]"""
                t = pool.tile([128, 128], F16, tag=tag)
                nc.gpsimd.dma_gather(
                    out_ap=t[:].rearrange("p (a n) -> p a n", a=1),
                    in_ap=table_ap,
                    idxs_ap=idx_sb[:, q * 8:(q + 1) * 8],
                    num_idxs=128, num_idxs_reg=128, elem_size=128,
                    transpose=False)
                return t

            def elu(pool, y, p, w, tag):
                neg = pool.tile([p, w], F32, tag=tag + "n")
                nc.vector.tensor_scalar_min(neg[:], y[:], 0.0)
                ee = pool.tile([p, w], F32, tag=tag + "e")
                nc.scalar.activation(ee[:], neg[:], AF.Exp)
                rel = pool.tile([p, w], F32, tag=tag + "r")
                nc.vector.tensor_relu(rel[:], y[:])
                z = pool.tile([p, w], F32, tag=tag + "z")
                nc.vector.scalar_tensor_tensor(z[:], ee[:], -1.0, rel[:],
                                               op0=ALU.add, op1=ALU.add)
                return z

            # ---------------- phase 0: ad1 tables (both encoders) ----------
            with (
                tc.tile_pool(name="p0", bufs=2) as sp,
                tc.tile_pool(name="p0ps", bufs=2, space="PSUM") as pp,
            ):
                for t in range(NTILES):
                    xoT = gath_t(sp, x16d[:], own1s, t, "xoT")
                    for e in ENCS:
                        ps = pp.tile([128, 32], F32, tag="ps", name="ps")
                        nc.tensor.matmul(ps[:], lhsT=xoT[:], rhs=wad1s[e][:],
                                         start=True, stop=True)
                        stg = sp.tile([128, 128], F16, tag="stg", name="stg")
                        nc.scalar.activation(stg[:, 0:32], ps[:], AF.Copy)
                        nc.vector.memset(stg[:, 32:128], 0)
                        nc.sync.dma_start(ad1[e][t * 128:(t + 1) * 128, :],
                                          stg[:])

            # ---------------- GAT layer ------------------------------------
            def gat_layer(e, layer):
                if layer == 1:
                    W, cw, out_col = 2048, C1, 0
                    wsb, adt = w1s[e], ad1[e]
                    table, idxs = x16d[:], srcgs[e]
                    brs = b1rs[e]
                else:
                    W, cw, out_col = 1024, C2, 64
                    wsb, adt = w2s[e], ad2[e]
                    table, idxs = z1all[e][:], src2s[e]
                    brs = b2rs[e]
                npieces = W // 512
                with (
                    tc.tile_pool(name=f"g{e}{layer}", bufs=3) as sp,
                    tc.tile_pool(name=f"f{e}{layer}", bufs=2) as fp,
                    tc.tile_pool(name=f"h{e}{layer}", bufs=2, space="PSUM") as pph,
                    tc.tile_pool(name=f"a{e}{layer}", bufs=1, space="PSUM") as ppa,
                    tc.tile_pool(name=f"c{e}{layer}", bufs=1, space="PSUM") as ppc,
                ):
                    for t in range(NTILES):
                        acc = ppc.tile([128, W + 32], F32, tag="acc", name="acc")
                        for j in range(nch):
                            q = t * nch + j
                            xsT = gath_t(sp, table, idxs, q, "xsT")
                            adg = gath_r(sp, adt[:], dstls[e], q, "adg")
                            stq = sp.tile([128, 128], F16, tag="stq", name="stq")
                            nc.sync.dma_start(
                                stq[:], std[e][q * 128:(q + 1) * 128, :])
                            pas = ppa.tile([128, 32], F32, tag="pas", name="pas")
                            nc.tensor.matmul(pas[:], lhsT=xsT[:],
                                             rhs=wsb[:, W:W + 32],
                                             start=True, stop=True)
                            adf = sp.tile([128, 32], F32, tag="adf", name="adf")
                            nc.vector.tensor_copy(adf[:], adg[:, 0:32])
                            lg = sp.tile([128, 32], F32, tag="lg", name="lg")
                            nc.vector.tensor_add(lg[:], pas[:], adf[:])
                            e1 = sp.tile([128, 32], F32, tag="e1", name="e1")
                            nc.scalar.activation(e1[:], lg[:], AF.Exp)
                            e2 = sp.tile([128, 32], F32, tag="e2", name="e2")
                            nc.scalar.activation(e2[:], lg[:], AF.Exp,
                                                 scale=0.2)
                            Xg = sp.tile([128, W + 32], F16, tag="Xg", name="Xg")
                            nc.vector.tensor_max(Xg[:, W:W + 32], e1[:], e2[:])
                            gsl = Xg[:, W:W + 32]
                            for i in range(npieces):
                                psh = pph.tile([128, 512], F32, tag="psh", name="psh")
                                nc.tensor.matmul(
                                    psh[:], lhsT=xsT[:],
                                    rhs=wsb[:, 512 * i:512 * (i + 1)],
                                    start=True, stop=True)
                                xg_v = Xg[:, 512 * i:512 * (i + 1)].rearrange(
                                    "p (c h) -> p c h", h=H)
                                if i < npieces // 2:
                                    xu = sp.tile([128, 512], F16, tag="xu", name="xu")
                                    nc.scalar.activation(xu[:], psh[:], AF.Copy)
                                    nc.vector.tensor_mul(
                                        xg_v,
                                        xu[:].rearrange("p (c h) -> p c h", h=H),
                                        _bc(gsl, 16))
                                else:
                                    nc.vector.tensor_mul(
                                        xg_v,
                                        psh[:].rearrange("p (c h) -> p c h", h=H),
                                        _bc(gsl, 16))
                            first, last = (j == 0), (j == nch - 1)
                            for i in range(npieces):
                                nc.tensor.matmul(
                                    acc[:, 512 * i:512 * (i + 1)], lhsT=stq[:],
                                    rhs=Xg[:, 512 * i:512 * (i + 1)],
                                    start=first, stop=last,
                                    skip_group_check=True)
                            nc.tensor.matmul(
                                acc[:, W:W + 32], lhsT=stq[:],
                                rhs=Xg[:, W:W + 32],
                                start=first, stop=last, skip_group_check=True)
                        # ---- finalize tile t ----
                        ssb = fp.tile([128, 32], F32, tag="ssb", name="ssb")
                        nc.vector.tensor_scalar_max(ssb[:], acc[:, W:W + 32],
                                                    1e-30)
                        r = fp.tile([128, 32], F32, tag="r", name="r")
                        nc.vector.reciprocal(r[:], ssb[:])
                        tmp = fp.tile([128, W], F32, tag="tmp", name="tmp")
                        nc.vector.tensor_mul(
                            tmp[:].rearrange("p (c h) -> p c h", h=H),
                            acc[:, 0:W].rearrange("p (c h) -> p c h", h=H),
                            _bc(r[:], cw))
                        m = fp.tile([128, cw], F32, tag="m", name="m")
                        nc.vector.reduce_sum(
                            m[:], tmp[:].rearrange("p (c h) -> p c h", h=H),
                            axis=mybir.AxisListType.X)
                        y = fp.tile([128, cw], F32, tag="y", name="y")
                        nc.vector.tensor_add(y[:], m[:], brs[:, 0:cw])
                        z = elu(fp, y, 128, cw, "fz")
                        rows = slice(t * 128, (t + 1) * 128)
                        nc.sync.dma_start(
                            xshd[e][rows, out_col:out_col + cw], z[:])
                        if layer == 1:
                            z16 = fp.tile([128, 128], F16, tag="z16", name="z16")
                            nc.vector.tensor_copy(z16[:, 0:64], z[:])
                            nc.vector.memset(z16[:, 64:128], 0)
                            nc.sync.dma_start(z1own[e][rows, :], z16[:])

            def ad2_tables(e):
                with (
                    tc.tile_pool(name=f"d2{e}", bufs=2) as sp,
                    tc.tile_pool(name=f"d2p{e}", bufs=2, space="PSUM") as pp,
                ):
                    for t in range(NTILES):
                        zoT = gath_t(sp, z1all[e][:], own2s, t, "zoT")
                        ps = pp.tile([128, 32], F32, tag="ps", name="ps")
                        nc.tensor.matmul(ps[:], lhsT=zoT[:], rhs=wad2s[e][:],
                                         start=True, stop=True)
                        stg = sp.tile([128, 128], F16, tag="stg", name="stg")
                        nc.scalar.activation(stg[:, 0:32], ps[:], AF.Copy)
                        nc.vector.memset(stg[:, 32:128], 0)
                        nc.sync.dma_start(ad2[e][t * 128:(t + 1) * 128, :],
                                          stg[:])

            def allgather(e):
                nc.gpsimd.collective_compute(
                    "AllGather", ALU.bypass,
                    replica_groups=[list(range(NCORES))],
                    ins=[z1own[e].opt()], outs=[z1all[e].opt()])

            # ---------------- schedule -------------------------------------
            gat_layer("i", 1)
            allgather("i")
            gat_layer("o", 1)
            allgather("o")
            ad2_tables("i")
            gat_layer("i", 2)
            ad2_tables("o")
            gat_layer("o", 2)

            # ---------------- AE (feature-major) ---------------------------
            with (
                tc.tile_pool(name="ae", bufs=2) as sp,
                tc.tile_pool(name="aeps", bufs=1, space="PSUM") as pp,
            ):
                b1c = aevs[0:64, 0:1]
                s1c = aevs[0:64, 1:2]
                t1c = aevs[0:64, 2:3]
                b2c = aevs[0:32, 3:4]
                s2c = aevs[0:32, 4:5]
                t2c = aevs[0:32, 5:6]
                db1c = aevs[0:64, 6:7]
                db2c = aevs[0:128, 7:8]
                for t in range(NTILES):
                    rows = slice(t * 128, (t + 1) * 128)
                    xoT = gath_t(sp, x16d[:], own1s, t, "xoT")
                    u1 = pp.tile([64, 128], F32, tag="u1", name="u1")
                    nc.tensor.matmul(u1[:], lhsT=w1es[:], rhs=xoT[:],
                                     start=True, stop=True)
                    y1 = sp.tile([64, 128], F32, tag="y1", name="y1")
                    nc.scalar.activation(y1[:], u1[:], AF.Identity, bias=b1c)
                    e1z = elu(sp, y1, 64, 128, "a1")
                    z1T = sp.tile([64, 128], F32, tag="z1T", name="z1T")
                    nc.scalar.activation(z1T[:], e1z[:], AF.Identity,
                                         bias=t1c, scale=s1c)
                    z1T6 = sp.tile([64, 128], F16, tag="z1T6", name="z1T6")
                    nc.vector.tensor_copy(z1T6[:], z1T[:])

                    u2 = pp.tile([32, 128], F32, tag="u2", name="u2")
                    nc.tensor.matmul(u2[:], lhsT=w2es[:], rhs=z1T6[:],
                                     start=True, stop=True)
                    y2 = sp.tile([32, 128], F32, tag="y2", name="y2")
                    nc.scalar.activation(y2[:], u2[:], AF.Identity, bias=b2c)
                    e2z = elu(sp, y2, 32, 128, "a2")
                    z2T = sp.tile([32, 128], F32, tag="z2T", name="z2T")
                    nc.scalar.activation(z2T[:], e2z[:], AF.Identity,
                                         bias=t2c, scale=s2c)
                    z2T6 = sp.tile([32, 128], F16, tag="z2T6", name="z2T6")
                    nc.vector.tensor_copy(z2T6[:], z2T[:])

                    u3 = pp.tile([64, 128], F32, tag="u3", name="u3")
                    nc.tensor.matmul(u3[:], lhsT=dw1es[:], rhs=z2T6[:],
                                     start=True, stop=True)
                    y3 = sp.tile([64, 128], F32, tag="y3", name="y3")
                    nc.scalar.activation(y3[:], u3[:], AF.Identity, bias=db1c)
                    d1 = elu(sp, y3, 64, 128, "a3")
                    d16 = sp.tile([64, 128], F16, tag="d16", name="d16")
                    nc.vector.tensor_copy(d16[:], d1[:])

                    u4 = pp.tile([128, 128], F32, tag="u4", name="u4")
                    nc.tensor.matmul(u4[:], lhsT=dw2es[:], rhs=d16[:],
                                     start=True, stop=True)
                    deT = sp.tile([128, 128], F32, tag="deT", name="deT")
                    nc.scalar.activation(deT[:], u4[:], AF.Sigmoid, bias=db2c)

                    zcat = sp.tile([96, 128], F32, tag="zcat", name="zcat")
                    nc.vector.tensor_copy(zcat[0:64, :], z1T[:])
                    nc.vector.tensor_copy(zcat[64:96, :], z2T[:])
                    tp1 = pp.tile([128, 96], F32, tag="tp1", name="tp1")
                    nc.tensor.transpose(tp1[:], zcat[:], idents[0:96, 0:96])
                    o1 = sp.tile([128, 96], F32, tag="o1", name="o1")
                    nc.scalar.activation(o1[:], tp1[:], AF.Copy)
                    nc.sync.dma_start(xselfd[rows, :], o1[:])
                    tp2 = pp.tile([128, 128], F32, tag="tp2", name="tp2")
                    nc.tensor.transpose(tp2[:], deT[:], idents[:])
                    o2 = sp.tile([128, 128], F32, tag="o2", name="o2")
                    nc.scalar.activation(o2[:], tp2[:], AF.Copy)
                    nc.sync.dma_start(zred[rows, :], o2[:])

    nc.compile()
    return nc


# ------------------------------------------------------------------- driver

def kernel(x, edge_index, params):
    global LAST_RESULT
    nch, shared, percore = _prep_inputs(x, edge_index, params)
    if nch not in _cache:
        _cache[nch] = _build(nch)
    nc = _cache[nch]
    in_maps = [{**shared, **percore[k]} for k in range(NCORES)]
    res = run_bass_kernel_spmd(nc, in_maps, core_ids=list(range(NCORES)))
    LAST_RESULT = res
    outs = res.results

    def gather_shards(name, width):
        return np.concatenate(
            [outs[k][name][0:NP, 0:width] for k in range(NCORES)], 0)

    x_in = gather_shards("xin_sh", 96)
    x_out = gather_shards("xout_sh", 96)
    x_self = gather_shards("xself_sh", 96)
    z_re = gather_shards("zre_sh", 128)
    return (x_in, x_out, x_self, z_re)


# revision 12
# speedup vs baseline: 1.1020x; 1.1020x over previous
"""Trainium2 Bass kernel for nn_Encoder_84069689852144 (GAT encoder pair + AE).

Self-contained: takes FULL inputs, shards across 8 NeuronCores internally,
returns FULL outputs (x_in, x_out, x_self, z_self_re).

Strategy (per core, SPMD one program, per-core data via in_maps):
  - Destination-node sharding: core k owns dst nodes [k*1250, (k+1)*1250),
    edges partitioned by dst, sorted, bucketed into 128-edge chunks per
    128-dst-node tile (host side).
  - Phase 0 builds a per-node projection table proj[n] = [a_s_in | a_s_out |
    a_d_in | a_d_out] (fp16, 256B rows); per-edge values come via dma_gather
    by src/dst index - no per-chunk projection matmuls.
  - GAT layer: per chunk, dma_gather(transpose) pulls x[src]^T as a ready
    matmul lhsT; PE computes h into PSUM; attention weight g = exp(leaky(
    a_s+a_d)) (max-free softmax, exactly equivalent) in per-tile batched ops;
    ACT/DVE evacuate+scale to fp16 Xg pieces; PE scatter-adds via host-built
    0/1 selection matrices S^T into a PSUM accumulator per dst tile;
    finalize divides by the segment sum, head-means (1/H folded into W),
    biases, ELU.
  - z1 rows carry [z1 | a_s2 | a_d2] (computed in a small post-L1 phase)
    through the AllGather so layer 2 gathers everything it needs.
  - AE runs feature-major, PE-transposed at the end to node-major.
"""
import os

import numpy as np

import concourse.bass as bass
import concourse.bacc as bacc
import concourse.mybir as mybir
import concourse.tile as tile
from concourse.bass_utils import run_bass_kernel_spmd

N = 10000
INF = 128
H = 32
C1 = 64
C2 = 32
NCORES = 8
NP = N // NCORES          # 1250 own dst nodes per core
NPAD = 1280               # padded own rows (10 tiles of 128)
NTILES = NPAD // 128
NALL = 10112              # all nodes padded to 79 tiles
BN_EPS = 1e-5

F16 = mybir.dt.float16
F32 = mybir.dt.float32
I16 = mybir.dt.int16
AF = mybir.ActivationFunctionType
ALU = mybir.AluOpType
ENCS = ("i", "o")

_cache: dict = {}
LAST_RESULT = None        # BassKernelResults of the most recent run


# ----------------------------------------------------------------- host prep

def _perm_cols(c):
    """new col (cc*H + h) <- old col (h*c + cc)  (c-major layout)"""
    cc, hh = np.meshgrid(np.arange(c), np.arange(H), indexing="ij")
    return (hh * c + cc).reshape(-1)


def _fold_gat(p, c):
    W = np.asarray(p["W"], np.float64)
    a_s = np.asarray(p["a_src"], np.float64)
    a_d = np.asarray(p["a_dst"], np.float64)
    Wr = W.reshape(W.shape[0], H, c)
    WAs = np.einsum("ihc,hc->ih", Wr, a_s)
    WAd = np.einsum("ihc,hc->ih", Wr, a_d)
    Wp = W[:, _perm_cols(c)] / H          # 1/H (head mean) folded in
    b = np.asarray(p["b"], np.float64)
    return Wp, WAs, WAd, b


def _wrap16(a):
    """[M] int array -> dma_gather idx layout [128, M//16] int16."""
    m = a.shape[0]
    w = a.reshape(m // 16, 16).T
    return np.tile(w, (8, 1)).astype(np.int16)


def _prep_edges(src, dst, nch=None):
    """Per-core edge tables for one encoder direction."""
    order = np.argsort(dst, kind="stable")
    s_s = src[order].astype(np.int64)
    d_s = dst[order].astype(np.int64)
    core = d_s // NP
    local = d_s - core * NP
    t = local // 128
    key = core * NTILES + t
    counts = np.bincount(key, minlength=NCORES * NTILES)
    nch_req = int(np.ceil(counts.max() / 128))
    if nch is None:
        nch = nch_req
    assert nch >= nch_req
    cap = nch * 128
    nchunks = NTILES * nch
    starts = np.zeros(NCORES * NTILES, np.int64)
    starts[1:] = np.cumsum(counts)[:-1]
    within = np.arange(len(order)) - starts[key]
    slot = (key % NTILES) * cap + within

    srcg = np.zeros((NCORES, NTILES * cap), np.int64)
    src2 = np.zeros_like(srcg)
    dstg = np.zeros_like(srcg)
    dstl2 = np.zeros_like(srcg)
    st = np.zeros((NCORES, nchunks * 128, 128), np.float16)
    srcg[core, slot] = s_s
    src2[core, slot] = (s_s // NP) * NPAD + s_s % NP
    dstg[core, slot] = d_s
    dstl2[core, slot] = core * NPAD + local
    st[core, slot, local - t * 128] = 1.0
    return nch, {
        "srcg": np.stack([_wrap16(srcg[k]) for k in range(NCORES)]),
        "src2": np.stack([_wrap16(src2[k]) for k in range(NCORES)]),
        "dstg": np.stack([_wrap16(dstg[k]) for k in range(NCORES)]),
        "dstl2": np.stack([_wrap16(dstl2[k]) for k in range(NCORES)]),
        "st": st,
    }


def _prep_inputs(x, edge_index, params):
    x32 = np.asarray(x, np.float32)
    ei = np.asarray(edge_index).astype(np.int64)
    shared = {"x16": x32.astype(np.float16)}

    wproj = np.zeros((128, 128), np.float64)   # [WAs_i|WAs_o|WAd_i|WAd_o]
    for col, (e, (p1, p2)) in enumerate(
            (("i", (params["gin1"], params["gin2"])),
             ("o", (params["gout1"], params["gout2"])))):
        Wp1, WAs1, WAd1, b1 = _fold_gat(p1, C1)
        Wp2, WAs2, WAd2, b2 = _fold_gat(p2, C2)
        shared[f"w1c_{e}"] = Wp1.astype(np.float16)           # [128, 2048]
        w2c = np.concatenate([Wp2, np.zeros((64, 1024))], 0)  # [128, 1024]
        shared[f"w2c_{e}"] = w2c.astype(np.float16)
        w2sa = np.concatenate(
            [np.concatenate([WAs2, WAd2], 1), np.zeros((64, 64))], 0)
        shared[f"w2sa_{e}"] = w2sa.astype(np.float16)         # [128, 64]
        wproj[:, col * 32:col * 32 + 32] = WAs1
        wproj[:, 64 + col * 32:96 + col * 32] = WAd1
        shared[f"b1r_{e}"] = np.broadcast_to(b1, (128, C1)).astype(np.float32)
        shared[f"b2r_{e}"] = np.broadcast_to(b2, (128, C2)).astype(np.float32)
    shared["wproj"] = wproj.astype(np.float16)

    ae = params["ae"]
    sbn = 1.0 / np.sqrt(1.0 + BN_EPS)
    shared["w1e"] = np.asarray(ae["w1"], np.float16)            # [128, 64]
    shared["w2e"] = np.asarray(ae["w2"], np.float16)            # [64, 32]
    shared["dw1e"] = np.asarray(ae["dw1"], np.float16)          # [32, 64]
    shared["dw2e"] = np.asarray(ae["dw2"], np.float16)          # [64, 128]
    aev = np.zeros((128, 8), np.float32)
    aev[0:64, 0] = np.asarray(ae["b1"])
    aev[0:64, 1] = np.asarray(ae["g1"]) * sbn
    aev[0:64, 2] = np.asarray(ae["be1"])
    aev[0:32, 3] = np.asarray(ae["b2"])
    aev[0:32, 4] = np.asarray(ae["g2"]) * sbn
    aev[0:32, 5] = np.asarray(ae["be2"])
    aev[0:64, 6] = np.asarray(ae["db1"])
    aev[0:128, 7] = np.asarray(ae["db2"])
    shared["aev"] = aev
    shared["ident"] = np.eye(128, dtype=np.float32)
    shared["allv"] = _wrap16(np.minimum(np.arange(NALL), N - 1))
    shared["ownl"] = _wrap16(np.arange(NPAD))

    nch_i, tab_i = _prep_edges(ei[0], ei[1])
    nch_o, tab_o = _prep_edges(ei[1], ei[0])
    nch = max(nch_i, nch_o)
    if nch_i < nch:
        _, tab_i = _prep_edges(ei[0], ei[1], nch)
    if nch_o < nch:
        _, tab_o = _prep_edges(ei[1], ei[0], nch)

    percore = []
    for k in range(NCORES):
        m = {}
        for e, tab in (("i", tab_i), ("o", tab_o)):
            m[f"srcg_{e}"] = tab["srcg"][k]
            m[f"src2_{e}"] = tab["src2"][k]
            m[f"dstg_{e}"] = tab["dstg"][k]
            m[f"dstl2_{e}"] = tab["dstl2"][k]
            m[f"st_{e}"] = tab["st"][k]
        own1 = k * NP + np.minimum(np.arange(NPAD), NP - 1)
        m["own1"] = _wrap16(own1)
        percore.append(m)
    return nch, shared, percore


# -------------------------------------------------------------- bass builder

def _bc(ap2d, rep):
    """[P, W] AP -> [P, rep, W] with step-0 (broadcast) middle dim."""
    return bass.AP(ap2d.tensor, ap2d.offset, [ap2d.ap[0], [0, rep], ap2d.ap[-1]])


def _build(nch, repeat=1, single_core=False):
    nchunks = NTILES * nch
    nc = bacc.Bacc("TRN2", target_bir_lowering=False, debug=False,
                   num_devices=1 if single_core else NCORES)

    def din(name, shape, dt):
        return nc.dram_tensor(name, shape, dt, kind="ExternalInput")

    x16d = din("x16", [N, INF], F16)
    w1cd = {e: din(f"w1c_{e}", [128, 2048], F16) for e in ENCS}
    w2cd = {e: din(f"w2c_{e}", [128, 1024], F16) for e in ENCS}
    w2sad = {e: din(f"w2sa_{e}", [128, 64], F16) for e in ENCS}
    wprojd = din("wproj", [128, 128], F16)
    b1rd = {e: din(f"b1r_{e}", [128, C1], F32) for e in ENCS}
    b2rd = {e: din(f"b2r_{e}", [128, C2], F32) for e in ENCS}
    w1ed = din("w1e", [128, 64], F16)
    w2ed = din("w2e", [64, 32], F16)
    dw1ed = din("dw1e", [32, 64], F16)
    dw2ed = din("dw2e", [64, 128], F16)
    aevd = din("aev", [128, 8], F32)
    identd = din("ident", [128, 128], F32)
    srcgd = {e: din(f"srcg_{e}", [128, nchunks * 8], I16) for e in ENCS}
    src2d = {e: din(f"src2_{e}", [128, nchunks * 8], I16) for e in ENCS}
    dstgd = {e: din(f"dstg_{e}", [128, nchunks * 8], I16) for e in ENCS}
    dstl2d = {e: din(f"dstl2_{e}", [128, nchunks * 8], I16) for e in ENCS}
    std = {e: din(f"st_{e}", [nchunks * 128, 128], F16) for e in ENCS}
    own1d = din("own1", [128, NTILES * 8], I16)
    allvd = din("allv", [128, (NALL // 128) * 8], I16)
    ownld = din("ownl", [128, NTILES * 8], I16)

    xshd = {"i": nc.dram_tensor("xin_sh", [NPAD, 96], F32, kind="ExternalOutput"),
            "o": nc.dram_tensor("xout_sh", [NPAD, 96], F32, kind="ExternalOutput")}
    xselfd = nc.dram_tensor("xself_sh", [NPAD, 96], F32, kind="ExternalOutput")
    zred = nc.dram_tensor("zre_sh", [NPAD, 128], F32, kind="ExternalOutput")

    with tile.TileContext(nc) as tc:
        with (
            tc.tile_pool(name="const", bufs=1) as cp,
            tc.tile_pool(name="dram", bufs=1, space="DRAM") as dp,
        ):
            def load_const(dram_t, shape, dt):
                t = cp.tile(shape, dt, tag=dram_t.name + "_sb",
                            name=dram_t.name + "_sb")
                nc.sync.dma_start(t[:], dram_t[:])
                return t

            w1s = {e: load_const(w1cd[e], [128, 2048], F16) for e in ENCS}
            w2s = {e: load_const(w2cd[e], [128, 1024], F16) for e in ENCS}
            w2sas = {e: load_const(w2sad[e], [128, 64], F16) for e in ENCS}
            wprojs = load_const(wprojd, [128, 128], F16)
            b1rs = {e: load_const(b1rd[e], [128, C1], F32) for e in ENCS}
            b2rs = {e: load_const(b2rd[e], [128, C2], F32) for e in ENCS}
            w1es = load_const(w1ed, [128, 64], F16)
            w2es = load_const(w2ed, [64, 32], F16)
            dw1es = load_const(dw1ed, [32, 64], F16)
            dw2es = load_const(dw2ed, [64, 128], F16)
            aevs = load_const(aevd, [128, 8], F32)
            idents = load_const(identd, [128, 128], F32)
            srcgs = {e: load_const(srcgd[e], [128, nchunks * 8], I16)
                     for e in ENCS}
            src2s = {e: load_const(src2d[e], [128, nchunks * 8], I16)
                     for e in ENCS}
            dstgs = {e: load_const(dstgd[e], [128, nchunks * 8], I16)
                     for e in ENCS}
            dstl2s = {e: load_const(dstl2d[e], [128, nchunks * 8], I16)
                      for e in ENCS}
            own1s = load_const(own1d, [128, NTILES * 8], I16)
            allvs = load_const(allvd, [128, (NALL // 128) * 8], I16)
            ownls = load_const(ownld, [128, NTILES * 8], I16)

            proj = dp.tile([NALL, 128], F16, tag="proj", name="proj")
            z1own = {e: dp.tile([NPAD, 128], F16, tag=f"z1own_{e}",
                                name=f"z1own_{e}") for e in ENCS}
            z1all = {}
            for _r in range(repeat):
                for e in ENCS:
                    z1all[(e, _r)] = dp.tile(
                        [NCORES * NPAD, 128], F16, tag=f"z1all_{e}{_r}",
                        name=f"z1all_{e}{_r}", addr_space="Shared")

            def gath_t(pool, table_ap, idx_sb, q, tag, nq=1):
                """gather-transpose nq*128 rows of 128 fp16 -> [128, nq*128]"""
                t = pool.tile([128, nq * 128], F16, tag=tag, name=tag)
                nc.gpsimd.dma_gather(
                    out_ap=t[:].rearrange("p (a n) -> p a n", a=1),
                    in_ap=table_ap,
                    idxs_ap=idx_sb[:, q * 8:(q + nq) * 8],
                    num_idxs=nq * 128, num_idxs_reg=nq * 128, elem_size=128,
                    transpose=True, single_packet=(nq * 128 <= 512))
                return t

            def gath_r(pool, table_ap, idx_sb, q, tag, nq=1):
                """plain gather: [:, j*128:(j+1)*128] is chunk j's rows"""
                t = pool.tile([128, nq * 128], F16, tag=tag, name=tag)
                nc.gpsimd.dma_gather(
                    out_ap=t[:].rearrange("p (j n) -> p j n", n=128),
                    in_ap=table_ap,
                    idxs_ap=idx_sb[:, q * 8:(q + nq) * 8],
                    num_idxs=nq * 128, num_idxs_reg=nq * 128, elem_size=128,
                    transpose=False, single_packet=(nq * 128 <= 512))
                return t

            def elu(pool, y, p, w, tag):
                neg = pool.tile([p, w], F32, tag=tag + "n", name=tag + "n")
                nc.vector.tensor_scalar_min(neg[:], y[:], 0.0)
                ee = pool.tile([p, w], F32, tag=tag + "e", name=tag + "e")
                nc.scalar.activation(ee[:], neg[:], AF.Exp)
                rel = pool.tile([p, w], F32, tag=tag + "r", name=tag + "r")
                nc.vector.tensor_relu(rel[:], y[:])
                z = pool.tile([p, w], F32, tag=tag + "z", name=tag + "z")
                nc.vector.scalar_tensor_tensor(z[:], ee[:], -1.0, rel[:],
                                               op0=ALU.add, op1=ALU.add)
                return z

            # ------- phase 0: per-node projection table (both encoders) ----
            with (
                tc.tile_pool(name="p0", bufs=3) as sp,
                tc.tile_pool(name="p0ps", bufs=2, space="PSUM") as pp,
            ):
                xaT = gath_t(sp, x16d[:], allvs, 0, "xaT", nq=NALL // 128)
                for t in range(NALL // 128):
                    ps = pp.tile([128, 128], F32, tag="ps", name="ps")
                    nc.tensor.matmul(ps[:], lhsT=xaT[:, t * 128:(t + 1) * 128],
                                     rhs=wprojs[:], start=True, stop=True)
                    stg = sp.tile([128, 128], F16, tag="stg", name="stg")
                    nc.scalar.activation(stg[:], ps[:], AF.Copy)
                    nc.sync.dma_start(proj[t * 128:(t + 1) * 128, :], stg[:])

            # ---------------- GAT layer ------------------------------------
            def gat_layer(e, layer, rep=0):
                if layer == 1:
                    W, cw, out_col = 2048, C1, 0
                    wsb = w1s[e]
                    table = x16d[:]
                    sa_tab = proj[:]
                    as_off = 0 if e == "i" else 32
                    ad_off = 64 if e == "i" else 96
                    idx_s, idx_d = srcgs[e], dstgs[e]
                    brs = b1rs[e]
                else:
                    W, cw, out_col = 1024, C2, 64
                    wsb = w2s[e]
                    table = z1all[(e, rep)][:]
                    sa_tab = z1all[(e, rep)][:]
                    as_off, ad_off = 64, 96
                    idx_s, idx_d = src2s[e], dstl2s[e]
                    brs = b2rs[e]
                npieces = W // 512
                with (
                    tc.tile_pool(name=f"g{e}{layer}r{rep}", bufs=3) as sp,
                    tc.tile_pool(name=f"f{e}{layer}r{rep}", bufs=2) as fp,
                    tc.tile_pool(name=f"h{e}{layer}r{rep}", bufs=3,
                                 space="PSUM") as pph,
                    tc.tile_pool(name=f"c{e}{layer}r{rep}", bufs=1,
                                 space="PSUM") as ppc,
                ):
                    for t in range(NTILES):
                        acc = ppc.tile([128, W + 32], F32, tag="acc",
                                       name="acc")
                        xsTt = gath_t(sp, table, idx_s, t * nch, "xsTt",
                                      nq=nch)
                        asgt = gath_r(sp, sa_tab, idx_s, t * nch, "asgt",
                                      nq=nch)
                        adgt = gath_r(sp, sa_tab, idx_d, t * nch, "adgt",
                                      nq=nch)
                        stt = sp.tile([128, nch * 128], F16, tag="stt",
                                      name="stt")
                        nc.sync.dma_start(
                            stt[:].rearrange("p (j n) -> p j n", n=128),
                            std[e][t * nch * 128:(t + 1) * nch * 128, :]
                            .rearrange("(j p) n -> p j n", p=128))
                        # batched attention weights for the whole tile
                        lgt = sp.tile([128, 32 * nch], F32, tag="lgt",
                                      name="lgt")
                        nc.vector.tensor_add(
                            lgt[:].rearrange("p (j n) -> p j n", n=32),
                            asgt[:].rearrange("p (j n) -> p j n",
                                              n=128)[:, :, as_off:as_off + 32],
                            adgt[:].rearrange("p (j n) -> p j n",
                                              n=128)[:, :, ad_off:ad_off + 32])
                        e1t = sp.tile([128, 32 * nch], F32, tag="e1t",
                                      name="e1t")
                        nc.scalar.activation(e1t[:], lgt[:], AF.Exp)
                        e2t = sp.tile([128, 32 * nch], F32, tag="e2t",
                                      name="e2t")
                        nc.scalar.activation(e2t[:], lgt[:], AF.Exp, scale=0.2)
                        g16 = sp.tile([128, 32 * nch], F16, tag="g16",
                                      name="g16")
                        nc.vector.tensor_max(g16[:], e1t[:], e2t[:])
                        for j in range(nch):
                            xsT = xsTt[:, j * 128:(j + 1) * 128]
                            stq = stt[:, j * 128:(j + 1) * 128]
                            gsl = g16[:, j * 32:(j + 1) * 32]
                            nact = 3 if npieces == 4 else 1
                            if npieces == 4 and j % 2 == 1:
                                nact = 2
                            first, last = (j == 0), (j == nch - 1)
                            for i in range(npieces):
                                psh = pph.tile([128, 512], F32, tag="psh",
                                               name="psh")
                                nc.tensor.matmul(
                                    psh[:], lhsT=xsT,
                                    rhs=wsb[:, 512 * i:512 * (i + 1)],
                                    start=True, stop=True)
                                xgp = sp.tile([128, 512], F16, tag=f"xgp{i}",
                                              name="xgp")
                                xg_v = xgp[:].rearrange(
                                    "p (c h) -> p c h", h=H)
                                if i < nact:
                                    xu = sp.tile([128, 512], F16, tag="xu",
                                                 name="xu")
                                    nc.scalar.activation(xu[:], psh[:],
                                                         AF.Copy)
                                    nc.vector.tensor_mul(
                                        xg_v,
                                        xu[:].rearrange("p (c h) -> p c h",
                                                        h=H),
                                        _bc(gsl, 16))
                                else:
                                    nc.vector.tensor_mul(
                                        xg_v,
                                        psh[:].rearrange("p (c h) -> p c h",
                                                         h=H),
                                        _bc(gsl, 16))
                                nc.tensor.matmul(
                                    acc[:, 512 * i:512 * (i + 1)], lhsT=stq,
                                    rhs=xgp[:],
                                    start=first, stop=last,
                                    skip_group_check=True)
                            nc.tensor.matmul(
                                acc[:, W:W + 32], lhsT=stq, rhs=gsl,
                                start=first, stop=last, skip_group_check=True)
                        # ---- finalize tile t ----
                        ssb = fp.tile([128, 32], F32, tag="ssb", name="ssb")
                        nc.vector.tensor_scalar_max(ssb[:], acc[:, W:W + 32],
                                                    1e-30)
                        r = fp.tile([128, 32], F32, tag="r", name="r")
                        nc.vector.reciprocal(r[:], ssb[:])
                        tmp = fp.tile([128, W], F32, tag="tmp", name="tmp")
                        nc.vector.tensor_mul(
                            tmp[:].rearrange("p (c h) -> p c h", h=H),
                            acc[:, 0:W].rearrange("p (c h) -> p c h", h=H),
                            _bc(r[:], cw))
                        m = fp.tile([128, cw], F32, tag="m", name="m")
                        nc.vector.reduce_sum(
                            m[:], tmp[:].rearrange("p (c h) -> p c h", h=H),
                            axis=mybir.AxisListType.X)
                        y = fp.tile([128, cw], F32, tag="y", name="y")
                        nc.vector.tensor_add(y[:], m[:], brs[:, 0:cw])
                        z = elu(fp, y, 128, cw, "fz")
                        rows = slice(t * 128, (t + 1) * 128)
                        nc.sync.dma_start(
                            xshd[e][rows, out_col:out_col + cw], z[:])
                        if layer == 1:
                            z16 = fp.tile([128, 128], F16, tag="z16",
                                          name="z16")
                            nc.vector.tensor_copy(z16[:, 0:64], z[:])
                            nc.vector.memset(z16[:, 64:128], 0)
                            nc.sync.dma_start(z1own[e][rows, :], z16[:])

            def sa2_tables(e, rep=0):
                """fill z1own cols 64:128 with [a_s2 | a_d2] before AG"""
                with (
                    tc.tile_pool(name=f"s2{e}r{rep}", bufs=2) as sp,
                    tc.tile_pool(name=f"s2p{e}r{rep}", bufs=2,
                                 space="PSUM") as pp,
                ):
                    zoTt = gath_t(sp, z1own[e][:], ownls, 0, "zoTt",
                                  nq=NTILES)
                    for t in range(NTILES):
                        ps = pp.tile([128, 64], F32, tag="ps", name="ps")
                        nc.tensor.matmul(
                            ps[:], lhsT=zoTt[:, t * 128:(t + 1) * 128],
                            rhs=w2sas[e][:], start=True, stop=True)
                        stg = sp.tile([128, 64], F16, tag="stg", name="stg")
                        nc.scalar.activation(stg[:], ps[:], AF.Copy)
                        nc.sync.dma_start(
                            z1own[e][t * 128:(t + 1) * 128, 64:128], stg[:])

            def allgather(e, rep=0):
                if single_core:
                    nc.sync.dma_start(z1all[(e, rep)][0:NPAD, :],
                                      z1own[e][:])
                    return
                nc.gpsimd.collective_compute(
                    "AllGather", ALU.bypass,
                    replica_groups=[list(range(NCORES))],
                    ins=[z1own[e].opt()], outs=[z1all[(e, rep)].opt()])

            # ---------------- schedule -------------------------------------
            for _rep in range(repeat):
                gat_layer("i", 1, _rep)
                sa2_tables("i", _rep)
                allgather("i", _rep)
                gat_layer("o", 1, _rep)
                sa2_tables("o", _rep)
                allgather("o", _rep)
                gat_layer("i", 2, _rep)
                gat_layer("o", 2, _rep)

            # ---------------- AE (feature-major) ---------------------------
            with (
                tc.tile_pool(name="ae", bufs=2) as sp,
                tc.tile_pool(name="aeps", bufs=1, space="PSUM") as pp,
            ):
                b1c = aevs[0:64, 0:1]
                s1c = aevs[0:64, 1:2]
                t1c = aevs[0:64, 2:3]
                b2c = aevs[0:32, 3:4]
                s2c = aevs[0:32, 4:5]
                t2c = aevs[0:32, 5:6]
                db1c = aevs[0:64, 6:7]
                db2c = aevs[0:128, 7:8]
                xoTa = gath_t(sp, x16d[:], own1s, 0, "xoTa", nq=NTILES)
                for t in range(NTILES):
                    rows = slice(t * 128, (t + 1) * 128)
                    xoT = xoTa[:, t * 128:(t + 1) * 128]
                    u1 = pp.tile([64, 128], F32, tag="u1", name="u1")
                    nc.tensor.matmul(u1[:], lhsT=w1es[:], rhs=xoT,
                                     start=True, stop=True)
                    y1 = sp.tile([64, 128], F32, tag="y1", name="y1")
                    nc.scalar.activation(y1[:], u1[:], AF.Identity, bias=b1c)
                    e1z = elu(sp, y1, 64, 128, "a1")
                    z1T = sp.tile([64, 128], F32, tag="z1T", name="z1T")
                    nc.scalar.activation(z1T[:], e1z[:], AF.Identity,
                                         bias=t1c, scale=s1c)
                    z1T6 = sp.tile([64, 128], F16, tag="z1T6", name="z1T6")
                    nc.vector.tensor_copy(z1T6[:], z1T[:])

                    u2 = pp.tile([32, 128], F32, tag="u2", name="u2")
                    nc.tensor.matmul(u2[:], lhsT=w2es[:], rhs=z1T6[:],
                                     start=True, stop=True)
                    y2 = sp.tile([32, 128], F32, tag="y2", name="y2")
                    nc.scalar.activation(y2[:], u2[:], AF.Identity, bias=b2c)
                    e2z = elu(sp, y2, 32, 128, "a2")
                    z2T = sp.tile([32, 128], F32, tag="z2T", name="z2T")
                    nc.scalar.activation(z2T[:], e2z[:], AF.Identity,
                                         bias=t2c, scale=s2c)
                    z2T6 = sp.tile([32, 128], F16, tag="z2T6", name="z2T6")
                    nc.vector.tensor_copy(z2T6[:], z2T[:])

                    u3 = pp.tile([64, 128], F32, tag="u3", name="u3")
                    nc.tensor.matmul(u3[:], lhsT=dw1es[:], rhs=z2T6[:],
                                     start=True, stop=True)
                    y3 = sp.tile([64, 128], F32, tag="y3", name="y3")
                    nc.scalar.activation(y3[:], u3[:], AF.Identity, bias=db1c)
                    d1 = elu(sp, y3, 64, 128, "a3")
                    d16 = sp.tile([64, 128], F16, tag="d16", name="d16")
                    nc.vector.tensor_copy(d16[:], d1[:])

                    u4 = pp.tile([128, 128], F32, tag="u4", name="u4")
                    nc.tensor.matmul(u4[:], lhsT=dw2es[:], rhs=d16[:],
                                     start=True, stop=True)
                    deT = sp.tile([128, 128], F32, tag="deT", name="deT")
                    nc.scalar.activation(deT[:], u4[:], AF.Sigmoid, bias=db2c)

                    zcat = sp.tile([96, 128], F32, tag="zcat", name="zcat")
                    nc.vector.tensor_copy(zcat[0:64, :], z1T[:])
                    nc.vector.tensor_copy(zcat[64:96, :], z2T[:])
                    tp1 = pp.tile([128, 96], F32, tag="tp1", name="tp1")
                    nc.tensor.transpose(tp1[:], zcat[:], idents[0:96, 0:96])
                    o1 = sp.tile([128, 96], F32, tag="o1", name="o1")
                    nc.scalar.activation(o1[:], tp1[:], AF.Copy)
                    nc.sync.dma_start(xselfd[rows, :], o1[:])
                    tp2 = pp.tile([128, 128], F32, tag="tp2", name="tp2")
                    nc.tensor.transpose(tp2[:], deT[:], idents[:])
                    o2 = sp.tile([128, 128], F32, tag="o2", name="o2")
                    nc.scalar.activation(o2[:], tp2[:], AF.Copy)
                    nc.sync.dma_start(zred[rows, :], o2[:])

    nc.compile()
    return nc


# ------------------------------------------------------------------- driver

def kernel(x, edge_index, params):
    global LAST_RESULT
    nch, shared, percore = _prep_inputs(x, edge_index, params)
    repeat = int(os.environ.get("K_REPEAT", "1"))
    key = (nch, repeat)
    if key not in _cache:
        _cache[key] = _build(nch, repeat)
    nc = _cache[key]
    in_maps = [{**shared, **percore[k]} for k in range(NCORES)]
    res = run_bass_kernel_spmd(nc, in_maps, core_ids=list(range(NCORES)))
    LAST_RESULT = res
    outs = res.results

    def gather_shards(name, width):
        return np.concatenate(
            [outs[k][name][0:NP, 0:width] for k in range(NCORES)], 0)

    x_in = gather_shards("xin_sh", 96)
    x_out = gather_shards("xout_sh", 96)
    x_self = gather_shards("xself_sh", 96)
    z_re = gather_shards("zre_sh", 128)
    return (x_in, x_out, x_self, z_re)


# revision 15
# speedup vs baseline: 1.2735x; 1.1556x over previous
"""Trainium2 Bass kernel for nn_Encoder_84069689852144 (GAT encoder pair + AE).

Self-contained: takes FULL inputs, shards across 8 NeuronCores internally,
returns FULL outputs (x_in, x_out, x_self, z_self_re).

Strategy (per core, SPMD one program, per-core data via in_maps):
  - Destination-node sharding: core k owns dst nodes [k*1250, (k+1)*1250),
    edges partitioned by dst, sorted, bucketed into 128-edge chunks per
    128-dst-node tile (host side).
  - Phase 0 builds a per-node projection table proj[n] = [a_s_in | a_s_out |
    a_d_in | a_d_out] (fp16, 256B rows); per-edge values come via dma_gather
    by src/dst index - no per-chunk projection matmuls.
  - GAT layer: per chunk, dma_gather(transpose) pulls x[src]^T as a ready
    matmul lhsT; PE computes h into PSUM; attention weight g = exp(leaky(
    a_s+a_d)) (max-free softmax, exactly equivalent) in per-tile batched ops;
    ACT/DVE evacuate+scale to fp16 Xg pieces; PE scatter-adds via host-built
    0/1 selection matrices S^T into a PSUM accumulator per dst tile;
    finalize divides by the segment sum, head-means (1/H folded into W),
    biases, ELU.
  - z1 rows carry [z1 | a_s2 | a_d2] (computed in a small post-L1 phase)
    through the AllGather so layer 2 gathers everything it needs.
  - AE runs feature-major, PE-transposed at the end to node-major.
"""
import os

import numpy as np

import concourse.bass as bass
import concourse.bacc as bacc
import concourse.mybir as mybir
import concourse.tile as tile
from concourse.bass_utils import run_bass_kernel_spmd

N = 10000
CFG = {"nact1": 3, "alt1": 0, "nact2": 1, "alt2": 1, "act_ev": 1,
       "psh1": 3, "acc2": 2}
INF = 128
H = 32
C1 = 64
C2 = 32
NCORES = 8
NP = N // NCORES          # 1250 own dst nodes per core
NPAD = 1280               # padded own rows (10 tiles of 128)
NTILES = NPAD // 128
NALL = 10112              # all nodes padded to 79 tiles
BN_EPS = 1e-5

F16 = mybir.dt.float16
F32 = mybir.dt.float32
I16 = mybir.dt.int16
AF = mybir.ActivationFunctionType
ALU = mybir.AluOpType
ENCS = ("i", "o")

_cache: dict = {}
LAST_RESULT = None        # BassKernelResults of the most recent run


# ----------------------------------------------------------------- host prep

def _perm_cols(c):
    """new col (cc*H + h) <- old col (h*c + cc)  (c-major layout)"""
    cc, hh = np.meshgrid(np.arange(c), np.arange(H), indexing="ij")
    return (hh * c + cc).reshape(-1)


def _fold_gat(p, c):
    W = np.asarray(p["W"], np.float64)
    a_s = np.asarray(p["a_src"], np.float64)
    a_d = np.asarray(p["a_dst"], np.float64)
    Wr = W.reshape(W.shape[0], H, c)
    WAs = np.einsum("ihc,hc->ih", Wr, a_s)
    WAd = np.einsum("ihc,hc->ih", Wr, a_d)
    Wp = W[:, _perm_cols(c)] / H          # 1/H (head mean) folded in
    b = np.asarray(p["b"], np.float64)
    return Wp, WAs, WAd, b


def _wrap16(a):
    """[M] int array -> dma_gather idx layout [128, M//16] int16."""
    m = a.shape[0]
    w = a.reshape(m // 16, 16).T
    return np.tile(w, (8, 1)).astype(np.int16)


def _balance(dst):
    """Degree-balanced node -> in-core position so tile edge counts are even.
    Returns pos[n] in [0, NPAD) within the owner core's shard."""
    pos = np.empty(N, np.int64)
    core_all = dst // NP
    local_all = dst - core_all * NP
    for k in range(NCORES):
        deg = np.bincount(local_all[core_all == k], minlength=NP)
        order = np.argsort(-deg, kind="stable")
        tile_edges = np.zeros(NTILES, np.int64)
        tile_nodes = np.zeros(NTILES, np.int64)
        slot = np.empty(NP, np.int64)
        for n in order:
            best, bv = -1, None
            for tt in range(NTILES):
                if tile_nodes[tt] < 128 and (bv is None
                                             or tile_edges[tt] < bv):
                    best, bv = tt, tile_edges[tt]
            slot[n] = best * 128 + tile_nodes[best]
            tile_nodes[best] += 1
            tile_edges[best] += deg[n]
        pos[k * NP:(k + 1) * NP] = slot
    return pos


def _prep_edges(src, dst, pos, pos_src, nch=None):
    """Per-core edge tables for one encoder direction.
    pos: dst-side node positions (this encoder's balance);
    pos_src: positions used for the z1 table (same encoder's balance)."""
    s_all = src.astype(np.int64)
    d_all = dst.astype(np.int64)
    core_a = d_all // NP
    pos_d_a = pos[d_all]
    key_a = core_a * NTILES + pos_d_a // 128
    order = np.argsort(key_a, kind="stable")
    s_s, d_s = s_all[order], d_all[order]
    core = core_a[order]
    pos_d = pos_d_a[order]
    tile = pos_d // 128
    slotin = pos_d % 128
    key = key_a[order]
    counts = np.bincount(key, minlength=NCORES * NTILES)
    nch_req = int(np.ceil(counts.max() / 128))
    if nch is None:
        nch = nch_req
    assert nch >= nch_req
    cap = nch * 128
    nchunks = NTILES * nch
    starts = np.zeros(NCORES * NTILES, np.int64)
    starts[1:] = np.cumsum(counts)[:-1]
    within = np.arange(len(order)) - starts[key]
    slot = tile * cap + within

    srcg = np.zeros((NCORES, NTILES * cap), np.int64)
    src2 = np.zeros_like(srcg)
    dstg = np.zeros_like(srcg)
    dstl2 = np.zeros_like(srcg)
    st = np.zeros((NCORES, nchunks * 128, 128), np.float16)
    srcg[core, slot] = s_s
    src2[core, slot] = (s_s // NP) * NPAD + pos_src[s_s]
    dstg[core, slot] = d_s
    dstl2[core, slot] = core * NPAD + pos_d
    st[core, slot, slotin] = 1.0
    return nch, {
        "srcg": np.stack([_wrap16(srcg[k]) for k in range(NCORES)]),
        "src2": np.stack([_wrap16(src2[k]) for k in range(NCORES)]),
        "dstg": np.stack([_wrap16(dstg[k]) for k in range(NCORES)]),
        "dstl2": np.stack([_wrap16(dstl2[k]) for k in range(NCORES)]),
        "st": st,
    }


def _prep_inputs(x, edge_index, params):
    x32 = np.asarray(x, np.float32)
    ei = np.asarray(edge_index).astype(np.int64)
    shared = {"x16": x32.astype(np.float16)}

    wproj = np.zeros((128, 128), np.float64)   # [WAs_i|WAs_o|WAd_i|WAd_o]
    for col, (e, (p1, p2)) in enumerate(
            (("i", (params["gin1"], params["gin2"])),
             ("o", (params["gout1"], params["gout2"])))):
        Wp1, WAs1, WAd1, b1 = _fold_gat(p1, C1)
        Wp2, WAs2, WAd2, b2 = _fold_gat(p2, C2)
        shared[f"w1c_{e}"] = Wp1.astype(np.float16)           # [128, 2048]
        w2c = np.concatenate([Wp2, np.zeros((64, 1024))], 0)  # [128, 1024]
        shared[f"w2c_{e}"] = w2c.astype(np.float16)
        w2sa = np.concatenate(
            [np.concatenate([WAs2, WAd2], 1), np.zeros((64, 64))], 0)
        shared[f"w2sa_{e}"] = w2sa.astype(np.float16)         # [128, 64]
        wproj[:, col * 32:col * 32 + 32] = WAs1
        wproj[:, 64 + col * 32:96 + col * 32] = WAd1
        shared[f"b1r_{e}"] = np.broadcast_to(b1, (128, C1)).astype(np.float32)
        shared[f"b2r_{e}"] = np.broadcast_to(b2, (128, C2)).astype(np.float32)
    shared["wproj"] = wproj.astype(np.float16)

    ae = params["ae"]
    sbn = 1.0 / np.sqrt(1.0 + BN_EPS)
    shared["w1e"] = np.asarray(ae["w1"], np.float16)            # [128, 64]
    shared["w2e"] = np.asarray(ae["w2"], np.float16)            # [64, 32]
    shared["dw1e"] = np.asarray(ae["dw1"], np.float16)          # [32, 64]
    shared["dw2e"] = np.asarray(ae["dw2"], np.float16)          # [64, 128]
    aev = np.zeros((128, 8), np.float32)
    aev[0:64, 0] = np.asarray(ae["b1"])
    aev[0:64, 1] = np.asarray(ae["g1"]) * sbn
    aev[0:64, 2] = np.asarray(ae["be1"])
    aev[0:32, 3] = np.asarray(ae["b2"])
    aev[0:32, 4] = np.asarray(ae["g2"]) * sbn
    aev[0:32, 5] = np.asarray(ae["be2"])
    aev[0:64, 6] = np.asarray(ae["db1"])
    aev[0:128, 7] = np.asarray(ae["db2"])
    shared["aev"] = aev
    shared["ident"] = np.eye(128, dtype=np.float32)
    shared["allv"] = _wrap16(np.minimum(np.arange(NALL), N - 1))
    shared["ownl"] = _wrap16(np.arange(NPAD))

    pos_i = _balance(ei[1])
    pos_o = _balance(ei[0])
    nch_i, tab_i = _prep_edges(ei[0], ei[1], pos_i, pos_i)
    nch_o, tab_o = _prep_edges(ei[1], ei[0], pos_o, pos_o)
    nch = max(nch_i, nch_o)
    if nch_i < nch:
        _, tab_i = _prep_edges(ei[0], ei[1], pos_i, pos_i, nch)
    if nch_o < nch:
        _, tab_o = _prep_edges(ei[1], ei[0], pos_o, pos_o, nch)
    shared["_pos_i"] = pos_i
    shared["_pos_o"] = pos_o

    percore = []
    for k in range(NCORES):
        m = {}
        for e, tab in (("i", tab_i), ("o", tab_o)):
            m[f"srcg_{e}"] = tab["srcg"][k]
            m[f"src2_{e}"] = tab["src2"][k]
            m[f"dstg_{e}"] = tab["dstg"][k]
            m[f"dstl2_{e}"] = tab["dstl2"][k]
            m[f"st_{e}"] = tab["st"][k]
        own1 = k * NP + np.minimum(np.arange(NPAD), NP - 1)
        m["own1"] = _wrap16(own1)
        percore.append(m)
    return nch, shared, percore


# -------------------------------------------------------------- bass builder

def _bc(ap2d, rep):
    """[P, W] AP -> [P, rep, W] with step-0 (broadcast) middle dim."""
    return bass.AP(ap2d.tensor, ap2d.offset, [ap2d.ap[0], [0, rep], ap2d.ap[-1]])


def _build(nch, repeat=1, single_core=False):
    nchunks = NTILES * nch
    nc = bacc.Bacc("TRN2", target_bir_lowering=False, debug=False,
                   num_devices=1 if single_core else NCORES)

    def din(name, shape, dt):
        return nc.dram_tensor(name, shape, dt, kind="ExternalInput")

    x16d = din("x16", [N, INF], F16)
    w1cd = {e: din(f"w1c_{e}", [128, 2048], F16) for e in ENCS}
    w2cd = {e: din(f"w2c_{e}", [128, 1024], F16) for e in ENCS}
    w2sad = {e: din(f"w2sa_{e}", [128, 64], F16) for e in ENCS}
    wprojd = din("wproj", [128, 128], F16)
    b1rd = {e: din(f"b1r_{e}", [128, C1], F32) for e in ENCS}
    b2rd = {e: din(f"b2r_{e}", [128, C2], F32) for e in ENCS}
    w1ed = din("w1e", [128, 64], F16)
    w2ed = din("w2e", [64, 32], F16)
    dw1ed = din("dw1e", [32, 64], F16)
    dw2ed = din("dw2e", [64, 128], F16)
    aevd = din("aev", [128, 8], F32)
    identd = din("ident", [128, 128], F32)
    srcgd = {e: din(f"srcg_{e}", [128, nchunks * 8], I16) for e in ENCS}
    src2d = {e: din(f"src2_{e}", [128, nchunks * 8], I16) for e in ENCS}
    dstgd = {e: din(f"dstg_{e}", [128, nchunks * 8], I16) for e in ENCS}
    dstl2d = {e: din(f"dstl2_{e}", [128, nchunks * 8], I16) for e in ENCS}
    std = {e: din(f"st_{e}", [nchunks * 128, 128], F16) for e in ENCS}
    own1d = din("own1", [128, NTILES * 8], I16)
    allvd = din("allv", [128, (NALL // 128) * 8], I16)
    ownld = din("ownl", [128, NTILES * 8], I16)

    xshd = {"i": nc.dram_tensor("xin_sh", [NPAD, 96], F32, kind="ExternalOutput"),
            "o": nc.dram_tensor("xout_sh", [NPAD, 96], F32, kind="ExternalOutput")}
    xselfd = nc.dram_tensor("xself_sh", [NPAD, 96], F32, kind="ExternalOutput")
    zred = nc.dram_tensor("zre_sh", [NPAD, 128], F32, kind="ExternalOutput")

    with tile.TileContext(nc) as tc:
        with (
            tc.tile_pool(name="const", bufs=1) as cp,
            tc.tile_pool(name="dram", bufs=1, space="DRAM") as dp,
        ):
            def load_const(dram_t, shape, dt):
                t = cp.tile(shape, dt, tag=dram_t.name + "_sb",
                            name=dram_t.name + "_sb")
                nc.sync.dma_start(t[:], dram_t[:])
                return t

            w1s = {e: load_const(w1cd[e], [128, 2048], F16) for e in ENCS}
            w2s = {e: load_const(w2cd[e], [128, 1024], F16) for e in ENCS}
            w2sas = {e: load_const(w2sad[e], [128, 64], F16) for e in ENCS}
            wprojs = load_const(wprojd, [128, 128], F16)
            b1rs = {e: load_const(b1rd[e], [128, C1], F32) for e in ENCS}
            b2rs = {e: load_const(b2rd[e], [128, C2], F32) for e in ENCS}
            w1es = load_const(w1ed, [128, 64], F16)
            w2es = load_const(w2ed, [64, 32], F16)
            dw1es = load_const(dw1ed, [32, 64], F16)
            dw2es = load_const(dw2ed, [64, 128], F16)
            aevs = load_const(aevd, [128, 8], F32)
            idents = load_const(identd, [128, 128], F32)
            srcgs = {e: load_const(srcgd[e], [128, nchunks * 8], I16)
                     for e in ENCS}
            src2s = {e: load_const(src2d[e], [128, nchunks * 8], I16)
                     for e in ENCS}
            dstgs = {e: load_const(dstgd[e], [128, nchunks * 8], I16)
                     for e in ENCS}
            dstl2s = {e: load_const(dstl2d[e], [128, nchunks * 8], I16)
                      for e in ENCS}
            own1s = load_const(own1d, [128, NTILES * 8], I16)
            allvs = load_const(allvd, [128, (NALL // 128) * 8], I16)
            ownls = load_const(ownld, [128, NTILES * 8], I16)

            proj = dp.tile([NALL, 128], F16, tag="proj", name="proj")
            z1own = {e: dp.tile([NPAD, 128], F16, tag=f"z1own_{e}",
                                name=f"z1own_{e}") for e in ENCS}
            z1all = {}
            for _r in range(repeat):
                for e in ENCS:
                    z1all[(e, _r)] = dp.tile(
                        [NCORES * NPAD, 128], F16, tag=f"z1all_{e}{_r}",
                        name=f"z1all_{e}{_r}", addr_space="Shared")

            def gath_t(pool, table_ap, idx_sb, q, tag, nq=1):
                """gather-transpose nq*128 rows of 128 fp16 -> [128, nq*128]"""
                t = pool.tile([128, nq * 128], F16, tag=tag, name=tag)
                nc.gpsimd.dma_gather(
                    out_ap=t[:].rearrange("p (a n) -> p a n", a=1),
                    in_ap=table_ap,
                    idxs_ap=idx_sb[:, q * 8:(q + nq) * 8],
                    num_idxs=nq * 128, num_idxs_reg=nq * 128, elem_size=128,
                    transpose=True, single_packet=(nq * 128 <= 512))
                return t

            def gath_r(pool, table_ap, idx_sb, q, tag, nq=1):
                """plain gather: [:, j*128:(j+1)*128] is chunk j's rows"""
                t = pool.tile([128, nq * 128], F16, tag=tag, name=tag)
                nc.gpsimd.dma_gather(
                    out_ap=t[:].rearrange("p (j n) -> p j n", n=128),
                    in_ap=table_ap,
                    idxs_ap=idx_sb[:, q * 8:(q + nq) * 8],
                    num_idxs=nq * 128, num_idxs_reg=nq * 128, elem_size=128,
                    transpose=False, single_packet=(nq * 128 <= 512))
                return t

            def elu(pool, y, p, w, tag):
                neg = pool.tile([p, w], F32, tag=tag + "n", name=tag + "n")
                nc.vector.tensor_scalar_min(neg[:], y[:], 0.0)
                ee = pool.tile([p, w], F32, tag=tag + "e", name=tag + "e")
                nc.scalar.activation(ee[:], neg[:], AF.Exp)
                rel = pool.tile([p, w], F32, tag=tag + "r", name=tag + "r")
                nc.vector.tensor_relu(rel[:], y[:])
                z = pool.tile([p, w], F32, tag=tag + "z", name=tag + "z")
                nc.vector.scalar_tensor_tensor(z[:], ee[:], -1.0, rel[:],
                                               op0=ALU.add, op1=ALU.add)
                return z

            # ------- phase 0: per-node projection table (both encoders) ----
            with (
                tc.tile_pool(name="p0", bufs=3) as sp,
                tc.tile_pool(name="p0ps", bufs=2, space="PSUM") as pp,
            ):
                xaT = gath_t(sp, x16d[:], allvs, 0, "xaT", nq=NALL // 128)
                for t in range(NALL // 128):
                    ps = pp.tile([128, 128], F32, tag="ps", name="ps")
                    nc.tensor.matmul(ps[:], lhsT=xaT[:, t * 128:(t + 1) * 128],
                                     rhs=wprojs[:], start=True, stop=True)
                    stg = sp.tile([128, 128], F16, tag="stg", name="stg")
                    nc.scalar.activation(stg[:], ps[:], AF.Copy)
                    nc.sync.dma_start(proj[t * 128:(t + 1) * 128, :], stg[:])

            # ---------------- GAT layer ------------------------------------
            def gat_layer(e, layer, rep=0):
                if layer == 1:
                    W, cw, out_col = 2048, C1, 0
                    wsb = w1s[e]
                    table = x16d[:]
                    sa_tab = proj[:]
                    as_off = 0 if e == "i" else 32
                    ad_off = 64 if e == "i" else 96
                    idx_s, idx_d = srcgs[e], dstgs[e]
                    brs = b1rs[e]
                else:
                    W, cw, out_col = 1024, C2, 64
                    wsb = w2s[e]
                    table = z1all[(e, rep)][:]
                    sa_tab = z1all[(e, rep)][:]
                    as_off, ad_off = 64, 96
                    idx_s, idx_d = src2s[e], dstl2s[e]
                    brs = b2rs[e]
                npieces = W // 512
                psh_bufs = CFG["psh1"] if layer == 1 else 2
                acc_bufs = 1 if layer == 1 else CFG["acc2"]
                with (
                    tc.tile_pool(name=f"g{e}{layer}r{rep}", bufs=3) as sp,
                    tc.tile_pool(name=f"f{e}{layer}r{rep}", bufs=2) as fp,
                    tc.tile_pool(name=f"h{e}{layer}r{rep}", bufs=psh_bufs,
                                 space="PSUM") as pph,
                    tc.tile_pool(name=f"c{e}{layer}r{rep}", bufs=acc_bufs,
                                 space="PSUM") as ppc,
                ):
                    for t in range(NTILES):
                        acc = ppc.tile([128, W + 32], F32, tag="acc",
                                       name="acc")
                        xsTt = gath_t(sp, table, idx_s, t * nch, "xsTt",
                                      nq=nch)
                        asgt = gath_r(sp, sa_tab, idx_s, t * nch, "asgt",
                                      nq=nch)
                        adgt = gath_r(sp, sa_tab, idx_d, t * nch, "adgt",
                                      nq=nch)
                        stt = sp.tile([128, nch * 128], F16, tag="stt",
                                      name="stt")
                        nc.sync.dma_start(
                            stt[:].rearrange("p (j n) -> p j n", n=128),
                            std[e][t * nch * 128:(t + 1) * nch * 128, :]
                            .rearrange("(j p) n -> p j n", p=128))
                        # batched attention weights for the whole tile
                        lgt = sp.tile([128, 32 * nch], F32, tag="lgt",
                                      name="lgt")
                        nc.vector.tensor_add(
                            lgt[:].rearrange("p (j n) -> p j n", n=32),
                            asgt[:].rearrange("p (j n) -> p j n",
                                              n=128)[:, :, as_off:as_off + 32],
                            adgt[:].rearrange("p (j n) -> p j n",
                                              n=128)[:, :, ad_off:ad_off + 32])
                        e1t = sp.tile([128, 32 * nch], F32, tag="e1t",
                                      name="e1t")
                        nc.scalar.activation(e1t[:], lgt[:], AF.Exp)
                        e2t = sp.tile([128, 32 * nch], F32, tag="e2t",
                                      name="e2t")
                        nc.scalar.activation(e2t[:], lgt[:], AF.Exp, scale=0.2)
                        g16 = sp.tile([128, 32 * nch], F16, tag="g16",
                                      name="g16")
                        nc.vector.tensor_max(g16[:], e1t[:], e2t[:])
                        for j in range(nch):
                            xsT = xsTt[:, j * 128:(j + 1) * 128]
                            stq = stt[:, j * 128:(j + 1) * 128]
                            gsl = g16[:, j * 32:(j + 1) * 32]
                            if npieces == 4:
                                nact = CFG["nact1"] + (CFG["alt1"] and j % 2)
                            else:
                                nact = CFG["nact2"] + (CFG["alt2"] and j % 2)
                            first, last = (j == 0), (j == nch - 1)
                            for i in range(npieces):
                                psh = pph.tile([128, 512], F32, tag="psh",
                                               name="psh")
                                nc.tensor.matmul(
                                    psh[:], lhsT=xsT,
                                    rhs=wsb[:, 512 * i:512 * (i + 1)],
                                    start=True, stop=True)
                                xgp = sp.tile([128, 512], F16, tag=f"xgp{i}",
                                              name="xgp")
                                xg_v = xgp[:].rearrange(
                                    "p (c h) -> p c h", h=H)
                                if i < nact:
                                    xu = sp.tile([128, 512], F16, tag="xu",
                                                 name="xu")
                                    nc.scalar.activation(xu[:], psh[:],
                                                         AF.Copy)
                                    nc.vector.tensor_mul(
                                        xg_v,
                                        xu[:].rearrange("p (c h) -> p c h",
                                                        h=H),
                                        _bc(gsl, 16))
                                else:
                                    nc.vector.tensor_mul(
                                        xg_v,
                                        psh[:].rearrange("p (c h) -> p c h",
                                                         h=H),
                                        _bc(gsl, 16))
                                nc.tensor.matmul(
                                    acc[:, 512 * i:512 * (i + 1)], lhsT=stq,
                                    rhs=xgp[:],
                                    start=first, stop=last,
                                    skip_group_check=True)
                            nc.tensor.matmul(
                                acc[:, W:W + 32], lhsT=stq, rhs=gsl,
                                start=first, stop=last, skip_group_check=True)
                        # ---- finalize tile t (acc evacuated by ACT) --
                        if CFG["act_ev"]:
                            ev = fp.tile([128, W + 32], F32, tag="ev",
                                         name="ev")
                            nc.scalar.activation(ev[:], acc[:], AF.Copy)
                        else:
                            ev = acc
                        ssb = fp.tile([128, 32], F32, tag="ssb", name="ssb")
                        nc.vector.tensor_scalar_max(ssb[:], ev[:, W:W + 32],
                                                    1e-30)
                        r = fp.tile([128, 32], F32, tag="r", name="r")
                        nc.vector.reciprocal(r[:], ssb[:])
                        tmp = fp.tile([128, W], F32, tag="tmp", name="tmp")
                        nc.vector.tensor_mul(
                            tmp[:].rearrange("p (c h) -> p c h", h=H),
                            ev[:, 0:W].rearrange("p (c h) -> p c h", h=H),
                            _bc(r[:], cw))
                        m = fp.tile([128, cw], F32, tag="m", name="m")
                        nc.vector.reduce_sum(
                            m[:], tmp[:].rearrange("p (c h) -> p c h", h=H),
                            axis=mybir.AxisListType.X)
                        y = fp.tile([128, cw], F32, tag="y", name="y")
                        nc.vector.tensor_add(y[:], m[:], brs[:, 0:cw])
                        z = elu(fp, y, 128, cw, "fz")
                        rows = slice(t * 128, (t + 1) * 128)
                        nc.sync.dma_start(
                            xshd[e][rows, out_col:out_col + cw], z[:])
                        if layer == 1:
                            z16 = fp.tile([128, 128], F16, tag="z16",
                                          name="z16")
                            nc.vector.tensor_copy(z16[:, 0:64], z[:])
                            nc.vector.memset(z16[:, 64:128], 0)
                            nc.sync.dma_start(z1own[e][rows, :], z16[:])

            def sa2_tables(e, rep=0):
                """fill z1own cols 64:128 with [a_s2 | a_d2] before AG"""
                with (
                    tc.tile_pool(name=f"s2{e}r{rep}", bufs=2) as sp,
                    tc.tile_pool(name=f"s2p{e}r{rep}", bufs=2,
                                 space="PSUM") as pp,
                ):
                    zoTt = gath_t(sp, z1own[e][:], ownls, 0, "zoTt",
                                  nq=NTILES)
                    for t in range(NTILES):
                        ps = pp.tile([128, 64], F32, tag="ps", name="ps")
                        nc.tensor.matmul(
                            ps[:], lhsT=zoTt[:, t * 128:(t + 1) * 128],
                            rhs=w2sas[e][:], start=True, stop=True)
                        stg = sp.tile([128, 64], F16, tag="stg", name="stg")
                        nc.scalar.activation(stg[:], ps[:], AF.Copy)
                        nc.sync.dma_start(
                            z1own[e][t * 128:(t + 1) * 128, 64:128], stg[:])

            def allgather(e, rep=0):
                if single_core:
                    nc.sync.dma_start(z1all[(e, rep)][0:NPAD, :],
                                      z1own[e][:])
                    return
                nc.gpsimd.collective_compute(
                    "AllGather", ALU.bypass,
                    replica_groups=[list(range(NCORES))],
                    ins=[z1own[e].opt()], outs=[z1all[(e, rep)].opt()])

            # ---------------- schedule -------------------------------------
            for _rep in range(repeat):
                gat_layer("i", 1, _rep)
                sa2_tables("i", _rep)
                allgather("i", _rep)
                gat_layer("o", 1, _rep)
                sa2_tables("o", _rep)
                allgather("o", _rep)
                gat_layer("i", 2, _rep)
                gat_layer("o", 2, _rep)

            # ---------------- AE (feature-major) ---------------------------
            with (
                tc.tile_pool(name="ae", bufs=2) as sp,
                tc.tile_pool(name="aeps", bufs=1, space="PSUM") as pp,
            ):
                b1c = aevs[0:64, 0:1]
                s1c = aevs[0:64, 1:2]
                t1c = aevs[0:64, 2:3]
                b2c = aevs[0:32, 3:4]
                s2c = aevs[0:32, 4:5]
                t2c = aevs[0:32, 5:6]
                db1c = aevs[0:64, 6:7]
                db2c = aevs[0:128, 7:8]
                xoTa = gath_t(sp, x16d[:], own1s, 0, "xoTa", nq=NTILES)
                for t in range(NTILES):
                    rows = slice(t * 128, (t + 1) * 128)
                    xoT = xoTa[:, t * 128:(t + 1) * 128]
                    u1 = pp.tile([64, 128], F32, tag="u1", name="u1")
                    nc.tensor.matmul(u1[:], lhsT=w1es[:], rhs=xoT,
                                     start=True, stop=True)
                    y1 = sp.tile([64, 128], F32, tag="y1", name="y1")
                    nc.scalar.activation(y1[:], u1[:], AF.Identity, bias=b1c)
                    e1z = elu(sp, y1, 64, 128, "a1")
                    z1T = sp.tile([64, 128], F32, tag="z1T", name="z1T")
                    nc.scalar.activation(z1T[:], e1z[:], AF.Identity,
                                         bias=t1c, scale=s1c)
                    z1T6 = sp.tile([64, 128], F16, tag="z1T6", name="z1T6")
                    nc.vector.tensor_copy(z1T6[:], z1T[:])

                    u2 = pp.tile([32, 128], F32, tag="u2", name="u2")
                    nc.tensor.matmul(u2[:], lhsT=w2es[:], rhs=z1T6[:],
                                     start=True, stop=True)
                    y2 = sp.tile([32, 128], F32, tag="y2", name="y2")
                    nc.scalar.activation(y2[:], u2[:], AF.Identity, bias=b2c)
                    e2z = elu(sp, y2, 32, 128, "a2")
                    z2T = sp.tile([32, 128], F32, tag="z2T", name="z2T")
                    nc.scalar.activation(z2T[:], e2z[:], AF.Identity,
                                         bias=t2c, scale=s2c)
                    z2T6 = sp.tile([32, 128], F16, tag="z2T6", name="z2T6")
                    nc.vector.tensor_copy(z2T6[:], z2T[:])

                    u3 = pp.tile([64, 128], F32, tag="u3", name="u3")
                    nc.tensor.matmul(u3[:], lhsT=dw1es[:], rhs=z2T6[:],
                                     start=True, stop=True)
                    y3 = sp.tile([64, 128], F32, tag="y3", name="y3")
                    nc.scalar.activation(y3[:], u3[:], AF.Identity, bias=db1c)
                    d1 = elu(sp, y3, 64, 128, "a3")
                    d16 = sp.tile([64, 128], F16, tag="d16", name="d16")
                    nc.vector.tensor_copy(d16[:], d1[:])

                    u4 = pp.tile([128, 128], F32, tag="u4", name="u4")
                    nc.tensor.matmul(u4[:], lhsT=dw2es[:], rhs=d16[:],
                                     start=True, stop=True)
                    deT = sp.tile([128, 128], F32, tag="deT", name="deT")
                    nc.scalar.activation(deT[:], u4[:], AF.Sigmoid, bias=db2c)

                    zcat = sp.tile([96, 128], F32, tag="zcat", name="zcat")
                    nc.vector.tensor_copy(zcat[0:64, :], z1T[:])
                    nc.vector.tensor_copy(zcat[64:96, :], z2T[:])
                    tp1 = pp.tile([128, 96], F32, tag="tp1", name="tp1")
                    nc.tensor.transpose(tp1[:], zcat[:], idents[0:96, 0:96])
                    o1 = sp.tile([128, 96], F32, tag="o1", name="o1")
                    nc.scalar.activation(o1[:], tp1[:], AF.Copy)
                    nc.sync.dma_start(xselfd[rows, :], o1[:])
                    tp2 = pp.tile([128, 128], F32, tag="tp2", name="tp2")
                    nc.tensor.transpose(tp2[:], deT[:], idents[:])
                    o2 = sp.tile([128, 128], F32, tag="o2", name="o2")
                    nc.scalar.activation(o2[:], tp2[:], AF.Copy)
                    nc.sync.dma_start(zred[rows, :], o2[:])

    nc.compile()
    return nc


# ------------------------------------------------------------------- driver

def kernel(x, edge_index, params):
    global LAST_RESULT
    nch, shared, percore = _prep_inputs(x, edge_index, params)
    repeat = int(os.environ.get("K_REPEAT", "1"))
    key = (nch, repeat)
    if key not in _cache:
        _cache[key] = _build(nch, repeat)
    nc = _cache[key]
    pos = {"i": shared.pop("_pos_i"), "o": shared.pop("_pos_o")}
    in_maps = [{**shared, **percore[k]} for k in range(NCORES)]
    res = run_bass_kernel_spmd(nc, in_maps, core_ids=list(range(NCORES)))
    LAST_RESULT = res
    outs = res.results

    def gather_shards(name, width, pos_e=None):
        blocks = []
        for k in range(NCORES):
            blk = outs[k][name]
            if pos_e is not None:
                blk = blk[pos_e[k * NP:(k + 1) * NP]]
            else:
                blk = blk[0:NP]
            blocks.append(blk[:, 0:width])
        return np.concatenate(blocks, 0)

    x_in = gather_shards("xin_sh", 96, pos["i"])
    x_out = gather_shards("xout_sh", 96, pos["o"])
    x_self = gather_shards("xself_sh", 96)
    z_re = gather_shards("zre_sh", 128)
    return (x_in, x_out, x_self, z_re)
